# Initial kernel scaffold
#
"""Bilateral blur (kornia-style, 5x5, L1 color distance squared) on 8 TRN2 cores.

Layout per core (one 1536x2048x3 fp32 image):
  - partition p (0..127) owns a 16-px-wide column stripe: img cols [16p, 16p+16)
  - each partition's SBUF rows hold a 20-px padded window: img cols [16p-2, 16p+18)
    interleaved (row, 20 px, 3 ch) -> 60 f32 per image row
  - rows processed in blocks of R=96 (+4 halo rows)
  - all 5x5 stencil shifts are free-dim offsets (no cross-partition traffic)

Math identical to the reference:
  t = sum_c |I(p+o) - I(p)|            (tensor_reduce add with apply_absolute_value)
  w = s_o * exp(-t^2/(2 sr^2))         (ACT: Square(A*t) then Exp(-u + ln s_o))
  out = clip(sum_o w*I(p+o) / sum_o w, 0, 1)   (1/den via ACT Ln+Exp, same table set)

Pair trick: the range term for offset +o at pixel p equals the range term for
-o at p+o, so t/w planes are computed once per unordered offset pair on a
slightly extended grid and read at two alignments.
"""

import numpy as np
from contextlib import ExitStack

import concourse.bass as bass
import concourse.bacc as bacc
import concourse.mybir as mybir
import concourse.tile as tile
from concourse.bass_utils import run_bass_kernel_spmd
from bass_rust import VecI64Pair

F32 = mybir.dt.float32

H, W, C = 1536, 2048, 3
NCORES = 8
KS = 5
SIGMA_S = 1.0
SIGMA_R = 0.06

ROWE = 60          # elems per image row per partition (20 px * 3 ch)
TCOL = 20          # t/w plane cols per partition


def _constants():
    x = (np.arange(KS, dtype=np.float32) - KS // 2).astype(np.float32)
    g = np.exp(-0.5 * (x / np.float32(SIGMA_S)) ** 2).astype(np.float32)
    g = g / g.sum()
    space = np.outer(g, g).astype(np.float32)  # [dy+2, dx+2]
    inv2sr2 = -0.5 / (SIGMA_R * SIGMA_R)       # -138.888..
    return space, inv2sr2


SPACE, INV2SR2 = _constants()
A_SQ = float(np.sqrt(-INV2SR2))                # Square scale: u = (A*t)^2 = |inv|*t^2
S_CENTER = float(SPACE[2, 2])

# canonical offset pairs (dy, dx): dy>0, or dy==0 and dx>0
PAIRS = [(0, 1), (0, 2)] + [(dy, dx) for dy in (1, 2) for dx in (-2, -1, 0, 1, 2)]


def _fview(ap2d, off, dims):
    """AP with the partition dim of `ap2d` and explicit free dims/offset (elems)."""
    v = ap2d.copy()
    v.offset = v.offset + off
    pdim = list(v.ap)[0]
    v.ap = VecI64Pair([list(pdim)] + [list(d) for d in dims])
    return v


def _dview(dram_ap, off, dims):
    v = dram_ap.copy()
    v.offset = v.offset + off
    v.ap = VecI64Pair([list(d) for d in dims])
    return v


def build_nc(h=H, r=96):
    nb = h // r
    assert h % r == 0
    rowlen = W * C  # 6144

    nc = bacc.Bacc("TRN2", target_bir_lowering=False, debug=False)
    img = nc.declare_dram_parameter("images", [h, W, C], F32, isOutput=False)
    out = nc.declare_dram_parameter("out", [h, W, C], F32, isOutput=True)
    img_a = img[:]
    out_a = out[:]

    with tile.TileContext(nc) as tc, ExitStack() as ctx:
        cpool = ctx.enter_context(tc.tile_pool(name="consts", bufs=1))
        tpool = ctx.enter_context(tc.tile_pool(name="input", bufs=2))
        npool = ctx.enter_context(tc.tile_pool(name="num", bufs=2))
        dpool = ctx.enter_context(tc.tile_pool(name="den", bufs=2))
        spool = ctx.enter_context(tc.tile_pool(name="diff", bufs=2))
        wpool = ctx.enter_context(tc.tile_pool(name="tw", bufs=3))
        ppool = ctx.enter_context(tc.tile_pool(name="prod", bufs=2))

        consts = cpool.tile([128, 2 + len(PAIRS)], F32)
        ca = consts[:]
        nc.vector.memset(ca[:, 0:1], -1.0)
        nc.vector.memset(ca[:, 1:2], A_SQ)
        for i, (dy, dx) in enumerate(PAIRS):
            s = float(SPACE[dy + 2, dx + 2])
            nc.vector.memset(ca[:, 2 + i:3 + i], float(np.log(s)))
        neg1 = ca[:, 0:1]
        a_sq = ca[:, 1:2]

        def load_rows(ta, tile_r0, n, img_r0, sgn):
            rs = sgn * rowlen
            base = img_r0 * rowlen
            # partitions 1..126: full 60-elem padded rows (overlapping windows)
            nc.sync.dma_start(
                out=_fview(ta[1:127], tile_r0 * ROWE, [[ROWE, n], [1, 60]]),
                in_=_dview(img_a, base + 42, [[48, 126], [rs, n], [1, 60]]),
            )
            # partition 0: main cols 0..17 -> elems 6..59
            nc.sync.dma_start(
                out=_fview(ta[0:1], tile_r0 * ROWE + 6, [[ROWE, n], [1, 54]]),
                in_=_dview(img_a, base + 0, [[0, 1], [rs, n], [1, 54]]),
            )
            # partition 0: reflect px 2,1 -> elems 0..5
            nc.sync.dma_start(
                out=_fview(ta[0:1], tile_r0 * ROWE, [[ROWE, n], [3, 2], [1, 3]]),
                in_=_dview(img_a, base + 6, [[0, 1], [rs, n], [-3, 2], [1, 3]]),
            )
            # partition 127: main cols 2030..2047 -> elems 0..53
            nc.sync.dma_start(
                out=_fview(ta[127:128], tile_r0 * ROWE, [[ROWE, n], [1, 54]]),
                in_=_dview(img_a, base + 6090, [[0, 1], [rs, n], [1, 54]]),
            )
            # partition 127: reflect px 2046,2045 -> elems 54..59
            nc.sync.dma_start(
                out=_fview(ta[127:128], tile_r0 * ROWE + 54, [[ROWE, n], [3, 2], [1, 3]]),
                in_=_dview(img_a, base + 6138, [[0, 1], [rs, n], [-3, 2], [1, 3]]),
            )

        for b in range(nb):
            r0 = b * r
            tin = tpool.tile([128, (r + 4) * ROWE], F32)
            ta = tin[:]
            if nb == 1:
                load_rows(ta, 2, r, 0, 1)
                load_rows(ta, 0, 2, 2, -1)
                load_rows(ta, r + 2, 2, h - 2, -1)
            elif b == 0:
                load_rows(ta, 2, r + 2, 0, 1)
                load_rows(ta, 0, 2, 2, -1)
            elif b == nb - 1:
                load_rows(ta, 0, r + 2, r0 - 2, 1)
                load_rows(ta, r + 2, 2, h - 2, -1)
            else:
                load_rows(ta, 0, r + 4, r0 - 2, 1)

            num = npool.tile([128, r * 48], F32)
            den = dpool.tile([128, r * 16], F32)
            na = num[:]
            da = den[:]

            # center tap: num = s_c * I, den = s_c
            ctr = _fview(ta, 2 * ROWE + 6, [[ROWE, r], [1, 48]])
            nc.vector.tensor_scalar(
                _fview(na, 0, [[48, r], [1, 48]]), ctr, S_CENTER, None,
                mybir.AluOpType.mult)
            nc.gpsimd.memset(da, S_CENTER)

            for i, (dy, dx) in enumerate(PAIRS):
                # extended grid q: rows [-dy, r), cols [-max(dx,0), 16+|dx|)
                qr0 = -dy
                nqr = r + dy
                qc0 = -max(dx, 0)
                nqc = 16 + abs(dx)
                dt_ = spool.tile([128, (r + 2) * ROWE], F32)
                tw = wpool.tile([128, (r + 2) * TCOL], F32)
                dv = dt_[:]
                wv = tw[:]

                # d = I(q+o) - I(q) over channels
                o_d = (qr0 + 2) * ROWE + (qc0 + 2) * 3
                nc.vector.tensor_tensor(
                    _fview(dv, o_d, [[ROWE, nqr], [3, nqc], [1, 3]]),
                    _fview(ta, (qr0 + dy + 2) * ROWE + (qc0 + dx + 2) * 3,
                           [[ROWE, nqr], [3, nqc], [1, 3]]),
                    _fview(ta, o_d, [[ROWE, nqr], [3, nqc], [1, 3]]),
                    mybir.AluOpType.subtract)
                # t = sum_c |d|
                o_t = (qr0 + 2) * TCOL + (qc0 + 2)
                tq = _fview(wv, o_t, [[TCOL, nqr], [1, nqc]])
                nc.vector.tensor_reduce(
                    tq,
                    _fview(dv, o_d, [[ROWE, nqr], [3, nqc], [1, 3]]),
                    axis=mybir.AxisListType.X, op=mybir.AluOpType.add,
                    apply_absolute_value=True)
                # u = (A*t)^2 ; w = exp(-u + ln s_o)
                nc.scalar.activation(tq, tq, mybir.ActivationFunctionType.Square,
                                     scale=a_sq)
                nc.scalar.activation(tq, tq, mybir.ActivationFunctionType.Exp,
                                     bias=ca[:, 2 + i:3 + i], scale=neg1)

                for sg in (1, -1):
                    ody, odx = sg * dy, sg * dx
                    # neighbor pixels I(p+o), out pixels p in [0,r)x[0,16)
                    nb_ap = _fview(ta, (2 + ody) * ROWE + (2 + odx) * 3,
                                   [[ROWE, r], [3, 16], [1, 3]])
                    # w at q = p (sg=1) or q = p-o (sg=-1)
                    wq0 = (2 + min(ody, 0)) * TCOL + (2 + min(odx, 0))
                    w_b = _fview(wv, wq0, [[TCOL, r], [1, 16], [0, 3]])
                    w_f = _fview(wv, wq0, [[TCOL, r], [1, 16]])
                    prod = ppool.tile([128, r * 48], F32)
                    pa = prod[:]
                    nc.vector.tensor_tensor(
                        _fview(pa, 0, [[48, r], [3, 16], [1, 3]]),
                        nb_ap, w_b, mybir.AluOpType.mult)
                    nc.vector.tensor_tensor(na, na, pa, mybir.AluOpType.add)
                    nc.gpsimd.tensor_tensor(da, da, w_f, mybir.AluOpType.add)

            # out = clip(num / den, 0, 1); 1/den = exp(-ln(den))
            nc.scalar.activation(da, da, mybir.ActivationFunctionType.Ln)
            nc.scalar.activation(da, da, mybir.ActivationFunctionType.Exp,
                                 scale=neg1)
            nc.vector.tensor_tensor(
                _fview(na, 0, [[48, r], [3, 16], [1, 3]]),
                _fview(na, 0, [[48, r], [3, 16], [1, 3]]),
                _fview(da, 0, [[16, r], [1, 16], [0, 3]]),
                mybir.AluOpType.mult)
            nc.vector.tensor_scalar(na, na, 0.0, 1.0,
                                    mybir.AluOpType.max, mybir.AluOpType.min)
            nc.sync.dma_start(
                out=_dview(out_a, r0 * rowlen, [[48, 128], [rowlen, r], [1, 48]]),
                in_=_fview(na, 0, [[48, r], [1, 48]]),
            )
    return nc


_CACHE = {}


def _get_nc(h=H, r=96):
    key = (h, r)
    if key not in _CACHE:
        _CACHE[key] = build_nc(h, r)
    return _CACHE[key]


def kernel(images: np.ndarray) -> np.ndarray:
    assert images.shape == (NCORES, H, W, C), images.shape
    nc = _get_nc()
    in_maps = [{"images": np.ascontiguousarray(images[i], dtype=np.float32)}
               for i in range(NCORES)]
    res = run_bass_kernel_spmd(nc, in_maps, core_ids=list(range(NCORES)))
    return np.stack([res.results[i]["out"] for i in range(NCORES)], axis=0)


# revision 10
# speedup vs baseline: 1.0380x; 1.0380x over previous
"""Bilateral blur (kornia 5x5, L1 color distance squared) on 8 TRN2 cores.

Data-parallel: one 1536x2048x3 fp32 image per NeuronCore. Residual form
  out = clip(ctr + (sum_o w_o * d_o) / den, 0, 1),  d_o = I(p+o) - I(p)
with the pair symmetry d_{-o}(p) = -d_{+o}(p-o): each unordered offset pair's
diff/weight planes are computed once and read at two alignments.

Findings baked in:
  - GpSimd activity contends with DVE via the shared SBUF port pair and slows
    every DVE op 30-90% -> all tensor work stays on the Vector engine.
  - fp16 (10-bit mantissa) gives near-fp32 accuracy for the residual form:
    emulated max abs err ~3.5e-5, max rel ~1.8e-3. The weight w is scaled by
    512 (bias += ln 512) to stay clear of fp16's subnormal floor; the scale
    cancels exactly in resid/den.
  - d must be produced by an fp32 subtract from fp32 pixels (relative-error
    regime); quantizing pixels first turns the error absolute and blows up
    the exponent accuracy of borderline weights.
  - t accumulation in fp16 is fine; 16-bit tensor_tensor runs at 2x.

Per-partition layouts (partition p owns img cols [16p-2, 16p+18)):
  T     (R+4) x (20px x 3ch)  fp32 interleaved
  d,|d| (R+2) x (3ch x 20px)  fp16 planar
  t,w   (R+2) x 20            fp16
  prod/resid  R x (3ch x 16)  fp16 planar
  den   R x 16                fp16    r32  R x 16  fp32
  stage R x 48                fp32 interleaved (px,ch) for DMA out
"""

import numpy as np
from contextlib import ExitStack

import concourse.bass as bass
import concourse.bacc as bacc
import concourse.mybir as mybir
import concourse.tile as tile
from concourse.bass_utils import run_bass_kernel_spmd
from bass_rust import VecI64Pair

F32 = mybir.dt.float32
F16 = mybir.dt.float16

H, W, C = 1536, 2048, 3
NCORES = 8
KS = 5
SIGMA_S = 1.0
SIGMA_R = 0.06
ROWE = 60
TCOL = 20
WSCALE = 512.0


def _constants():
    x = (np.arange(KS, dtype=np.float32) - KS // 2).astype(np.float32)
    g = np.exp(-0.5 * (x / np.float32(SIGMA_S)) ** 2).astype(np.float32)
    g = g / g.sum()
    space = np.outer(g, g).astype(np.float32)
    inv2sr2 = -0.5 / (SIGMA_R * SIGMA_R)
    return space, inv2sr2


SPACE, INV2SR2 = _constants()
A_SQ = float(np.sqrt(-INV2SR2))
S_CENTER = float(SPACE[2, 2])
PAIRS = [(0, 1), (0, 2)] + [(dy, dx) for dy in (1, 2) for dx in (-2, -1, 0, 1, 2)]
# pairs whose spatial weight is small enough that fp16-quantized inputs to the
# subtract only perturb the output at the ~2e-3 relative tail level
SMALL_S = {(0, 2), (1, -2), (1, 2), (2, -2), (2, -1), (2, 0), (2, 1), (2, 2)}
T16_SUBS = False  # True: 3.90ms but elem-rel tail 2.8e-2; False: safer 6.8e-3 tail


def _fview(ap2d, off, dims):
    v = ap2d.copy()
    v.offset = v.offset + off
    pdim = list(v.ap)[0]
    v.ap = VecI64Pair([list(pdim)] + [list(d) for d in dims])
    return v


def _dview(dram_ap, off, dims):
    v = dram_ap.copy()
    v.offset = v.offset + off
    v.ap = VecI64Pair([list(d) for d in dims])
    return v


def _pin_act_table_set():
    """Force every activation onto natural_log_exp_and_others (it holds all of
    Abs/Square/Exp/Ln), instead of walrus ping-ponging between exp_and_others
    and natural_log around each block's Ln (2 table reloads per block).
    Other sets are emptied but keep their positions so act_func_set_id
    indices stay aligned with act_info.json."""
    import concourse.hw_specs as hw_specs
    import concourse.bacc as bacc_mod
    orig = hw_specs.get_activation_tables
    if getattr(bacc_mod.get_activation_tables, "_pinned", False):
        return

    def patched(arch):
        t = dict(orig(arch))
        keep = "natural_log_exp_and_others"
        if keep in t:
            t = {k: (v if k == keep else set()) for k, v in t.items()}
        return t

    patched._pinned = True
    bacc_mod.get_activation_tables = patched


def build_nc(h=H, r=96):
    _pin_act_table_set()
    nb_blocks = h // r
    assert h % r == 0
    rowlen = W * C

    nc = bacc.Bacc("TRN2", target_bir_lowering=False, debug=False)
    img = nc.declare_dram_parameter("images", [h, W, C], F32, isOutput=False)
    out = nc.declare_dram_parameter("out", [h, W, C], F32, isOutput=True)
    img_a = img[:]
    out_a = out[:]

    with tile.TileContext(nc) as tc, ExitStack() as ctx:
        cpool = ctx.enter_context(tc.tile_pool(name="consts", bufs=1))
        tpool = ctx.enter_context(tc.tile_pool(name="input", bufs=2))
        rpool = ctx.enter_context(tc.tile_pool(name="resid", bufs=2))
        dnpool = ctx.enter_context(tc.tile_pool(name="den", bufs=2))
        rcpool = ctx.enter_context(tc.tile_pool(name="recip", bufs=2))
        dpool = ctx.enter_context(tc.tile_pool(name="diff", bufs=2))
        apool = ctx.enter_context(tc.tile_pool(name="absd", bufs=2))
        ttpool = ctx.enter_context(tc.tile_pool(name="tplane", bufs=2))
        wpool = ctx.enter_context(tc.tile_pool(name="wplane", bufs=3))
        ppool = ctx.enter_context(tc.tile_pool(name="prod", bufs=2))
        gpool = ctx.enter_context(tc.tile_pool(name="stage", bufs=2))
        t16pool = ctx.enter_context(tc.tile_pool(name="t16", bufs=2))

        consts = cpool.tile([128, 2 + len(PAIRS)], F32)
        ca = consts[:]
        nc.vector.memset(ca[:, 0:1], -1.0)
        nc.vector.memset(ca[:, 1:2], A_SQ)
        for i, (dy, dx) in enumerate(PAIRS):
            s = float(SPACE[dy + 2, dx + 2])
            nc.vector.memset(ca[:, 2 + i:3 + i], float(np.log(s * WSCALE)))
        neg1 = ca[:, 0:1]
        a_sq = ca[:, 1:2]

        def load_rows(ta, tile_r0, n, img_r0, sgn):
            if sgn < 0:
                for i in range(n):
                    load_rows(ta, tile_r0 + i, 1, img_r0 - i, 1)
                return
            rs = rowlen
            base = img_r0 * rowlen
            nc.sync.dma_start(
                out=_fview(ta[1:127], tile_r0 * ROWE, [[ROWE, n], [1, 60]]),
                in_=_dview(img_a, base + 42, [[48, 126], [rs, n], [1, 60]]),
            )
            nc.sync.dma_start(
                out=_fview(ta[0:1], tile_r0 * ROWE + 6, [[ROWE, n], [1, 54]]),
                in_=_dview(img_a, base + 0, [[0, 1], [rs, n], [1, 54]]),
            )
            for do, so in ((0, 6), (3, 3)):
                nc.sync.dma_start(
                    out=_fview(ta[0:1], tile_r0 * ROWE + do, [[ROWE, n], [1, 3]]),
                    in_=_dview(img_a, base + so, [[0, 1], [rs, n], [1, 3]]),
                )
            nc.sync.dma_start(
                out=_fview(ta[127:128], tile_r0 * ROWE, [[ROWE, n], [1, 54]]),
                in_=_dview(img_a, base + 6090, [[0, 1], [rs, n], [1, 54]]),
            )
            for do, so in ((54, 6138), (57, 6135)):
                nc.sync.dma_start(
                    out=_fview(ta[127:128], tile_r0 * ROWE + do, [[ROWE, n], [1, 3]]),
                    in_=_dview(img_a, base + so, [[0, 1], [rs, n], [1, 3]]),
                )

        for b in range(nb_blocks):
            r0 = b * r
            tin = tpool.tile([128, (r + 4) * ROWE], F32)
            ta = tin[:]
            if nb_blocks == 1:
                load_rows(ta, 2, r, 0, 1)
                load_rows(ta, 0, 2, 2, -1)
                load_rows(ta, r + 2, 2, h - 2, -1)
            elif b == 0:
                load_rows(ta, 2, r + 2, 0, 1)
                load_rows(ta, 0, 2, 2, -1)
            elif b == nb_blocks - 1:
                load_rows(ta, 0, r + 2, r0 - 2, 1)
                load_rows(ta, r + 2, 2, h - 2, -1)
            else:
                load_rows(ta, 0, r + 4, r0 - 2, 1)

            t16 = None
            if T16_SUBS:
                # fp16 planar copy of T for the small-weight pairs' subtracts
                t16 = t16pool.tile([128, (r + 4) * ROWE], F16)
                nc.scalar.activation(
                    _fview(t16[:], 0, [[ROWE, r + 4], [TCOL, 3], [1, 20]]),
                    _fview(ta, 0, [[ROWE, r + 4], [1, 3], [3, 20]]),
                    mybir.ActivationFunctionType.Copy)

            resid = rpool.tile([128, r * 48], F16)
            den = dnpool.tile([128, r * 16], F16)
            ra = resid[:]
            da = den[:]
            first_resid = [True]
            first_den = [True]

            for i, (dy, dx) in enumerate(PAIRS):
                qr0 = -dy
                nqr = r + dy
                qc0 = -max(dx, 0)
                col_lo = qc0 + 2
                col_e = col_lo & ~1
                nqc = 16 + abs(dx) + (col_lo - col_e)
                ri0 = qr0 + 2

                dt_ = dpool.tile([128, (r + 2) * ROWE], F16)
                ad_ = apool.tile([128, (r + 2) * ROWE], F16)
                tt_ = ttpool.tile([128, (r + 2) * TCOL], F16)
                tw_ = wpool.tile([128, (r + 2) * TCOL], F16)
                dv, av, tv, wv = dt_[:], ad_[:], tt_[:], tw_[:]

                d_out = _fview(dv, ri0 * ROWE + col_e, [[ROWE, nqr], [TCOL, 3], [1, nqc]])
                if T16_SUBS and (dy, dx) in SMALL_S:
                    # fp16 2x subtract from the planar fp16 copy
                    nc.vector.tensor_tensor(
                        d_out,
                        _fview(t16[:], (ri0 + dy) * ROWE + col_e + dx,
                               [[ROWE, nqr], [TCOL, 3], [1, nqc]]),
                        _fview(t16[:], ri0 * ROWE + col_e,
                               [[ROWE, nqr], [TCOL, 3], [1, nqc]]),
                        mybir.AluOpType.subtract)
                else:
                    nc.vector.tensor_tensor(
                        d_out,
                        _fview(ta, (ri0 + dy) * ROWE + (col_e + dx) * 3,
                               [[ROWE, nqr], [1, 3], [3, nqc]]),
                        _fview(ta, ri0 * ROWE + col_e * 3,
                               [[ROWE, nqr], [1, 3], [3, nqc]]),
                        mybir.AluOpType.subtract)
                nc.scalar.activation(
                    _fview(av, ri0 * ROWE + col_e, [[ROWE, nqr], [TCOL, 3], [1, nqc]]),
                    d_out, mybir.ActivationFunctionType.Abs)
                tq = _fview(tv, ri0 * TCOL + col_e, [[TCOL, nqr], [1, nqc]])
                nc.vector.tensor_tensor(
                    tq,
                    _fview(av, ri0 * ROWE + 0 * TCOL + col_e, [[ROWE, nqr], [1, nqc]]),
                    _fview(av, ri0 * ROWE + 1 * TCOL + col_e, [[ROWE, nqr], [1, nqc]]),
                    mybir.AluOpType.add)
                nc.vector.tensor_tensor(
                    tq, tq,
                    _fview(av, ri0 * ROWE + 2 * TCOL + col_e, [[ROWE, nqr], [1, nqc]]),
                    mybir.AluOpType.add)
                nc.scalar.activation(tq, tq, mybir.ActivationFunctionType.Square,
                                     scale=a_sq)
                wq = _fview(wv, ri0 * TCOL + col_e, [[TCOL, nqr], [1, nqc]])
                nc.scalar.activation(wq, tq, mybir.ActivationFunctionType.Exp,
                                     bias=ca[:, 2 + i:3 + i], scale=neg1)

                prod = ppool.tile([128, r * 48], F16)
                pa = prod[:]
                for sg in (1, -1):
                    ri, ci = (2, 2) if sg == 1 else (2 - dy, 2 - dx)
                    for ch in range(3):
                        nc.vector.tensor_tensor(
                            _fview(pa, ch * 16, [[48, r], [1, 16]]),
                            _fview(dv, ri * ROWE + ch * TCOL + ci, [[ROWE, r], [1, 16]]),
                            _fview(wv, ri * TCOL + ci, [[TCOL, r], [1, 16]]),
                            mybir.AluOpType.mult)
                    if first_resid[0]:
                        assert sg == 1
                        nc.vector.tensor_copy(ra, pa)
                        first_resid[0] = False
                    else:
                        nc.vector.tensor_tensor(
                            ra, ra, pa,
                            mybir.AluOpType.add if sg == 1 else mybir.AluOpType.subtract)
                    wslice = _fview(wv, ri * TCOL + ci, [[TCOL, r], [1, 16]])
                    if first_den[0]:
                        nc.vector.tensor_scalar_add(da, wslice, S_CENTER * WSCALE)
                        first_den[0] = False
                    else:
                        nc.vector.tensor_tensor(da, da, wslice, mybir.AluOpType.add)

            # 1/den (x WSCALE, cancels): r32 = exp(-ln(den))
            r32 = rcpool.tile([128, r * 16], F32)
            rca = r32[:]
            nc.scalar.activation(rca, da, mybir.ActivationFunctionType.Ln)
            nc.scalar.activation(rca, rca, mybir.ActivationFunctionType.Exp,
                                 scale=neg1)
            stage = gpool.tile([128, r * 48], F32)
            sa = stage[:]
            for ch in range(3):
                nc.vector.tensor_tensor(
                    _fview(sa, ch, [[48, r], [3, 16]]),
                    _fview(ra, ch * 16, [[48, r], [1, 16]]),
                    _fview(rca, 0, [[16, r], [1, 16]]),
                    mybir.AluOpType.mult)
            nc.vector.tensor_tensor(
                sa, sa, _fview(ta, 2 * ROWE + 6, [[ROWE, r], [1, 48]]),
                mybir.AluOpType.add)
            nc.vector.tensor_scalar(sa, sa, 0.0, 1.0,
                                    mybir.AluOpType.max, mybir.AluOpType.min)
            nc.sync.dma_start(
                out=_dview(out_a, r0 * rowlen, [[48, 128], [rowlen, r], [1, 48]]),
                in_=_fview(sa, 0, [[48, r], [1, 48]]),
            )
    nc.finalize()
    return nc


_CACHE = {}


def _get_nc(h=H, r=96):
    key = (h, r)
    if key not in _CACHE:
        _CACHE[key] = build_nc(h, r)
    return _CACHE[key]


TRACE = False
LAST_RESULT = None


def kernel(images: np.ndarray) -> np.ndarray:
    global LAST_RESULT
    assert images.shape == (NCORES, H, W, C), images.shape
    nc = _get_nc()
    in_maps = [{"images": np.ascontiguousarray(images[i], dtype=np.float32)}
               for i in range(NCORES)]
    res = run_bass_kernel_spmd(nc, in_maps, core_ids=list(range(NCORES)),
                               trace=TRACE)
    LAST_RESULT = res
    return np.stack([res.results[i]["out"] for i in range(NCORES)], axis=0)
